# revision 28
# baseline (speedup 1.0000x reference)
"""Bass/Trainium2 kernel for additive-attention pooling.

    y = tanh(x @ W); s = y @ v; w = softmax(s, axis=T); out = w @ x

Shapes (full): x [16, 4096, 512] f32, att_W [512, 512] f32, att_v [512] f32
-> out [16, 512] f32.

Sharding: data-parallel over batch, 2 batches per core on 8 cores;
att_W / att_v replicated.

Per-core algorithm (streaming, single pass over x):
  for each 128-row tile of x:
    - GPSIMD cast-DMA: HBM fp32 -> SBUF bf16 (cast happens in the DMA)
    - PE transpose-mode matmuls put the D axis on partitions (PSUM bf16)
    - evacuate PSUM -> SBUF (alternating ScalarE/VectorE)
    - 4 accumulating bf16 matmuls vs resident W tiles -> y tile in PSUM
    - tanh on ScalarE (PSUM -> SBUF bf16)
    - scores col via fused multiply+reduce against broadcast v (DVE)
    - exp(scores col) on ScalarE (scores are ~N(0,0.05): no max needed)
    - accumulating M=1 matmul  e_col^T @ x_tile  -> unnormalized pooled sum
  per batch: den = sum(exp(scores)) via row-reduce + K=128 matmul with ones,
  out = num * (1/den).
"""

import numpy as np

B, T, D = 16, 4096, 512
N_CORES = 8
B_LOC = B // N_CORES          # batches per core
P = 128                       # partitions
TPB = T // P                  # 32 t-tiles per batch
KT = D // P                   # 4 contraction tiles
QUAD = 4                      # t-tiles per cast-DMA
POOL_LAG = 8                  # tiles of slack before a pooling matmul issues

_cached = {}


def _build(repeat=None):
    from contextlib import ExitStack

    import concourse.bass as bass
    import concourse.mybir as mybir
    from concourse import bacc
    from concourse.masks import make_identity
    from concourse.tile import TileContext

    f32 = mybir.dt.float32
    bf16 = mybir.dt.bfloat16
    AF = mybir.ActivationFunctionType
    ALU = mybir.AluOpType

    nc = bacc.Bacc("TRN2", target_bir_lowering=False, debug=False,
                   num_devices=N_CORES)
    x = nc.declare_dram_parameter("x", [B_LOC, T, D], f32, isOutput=False)
    att_W = nc.declare_dram_parameter("att_W", [D, D], f32, isOutput=False)
    att_v = nc.declare_dram_parameter("att_v", [D], f32, isOutput=False)
    out = nc.declare_dram_parameter("out", [B_LOC, D], f32, isOutput=True)

    with ExitStack() as ctx:
        tc = ctx.enter_context(TileContext(nc))
        singles = ctx.enter_context(tc.tile_pool(name="singles", bufs=1))
        # keep a whole batch of x tiles alive so the pooling matmuls can run
        # as one burst at batch end (no mid-stream PE stalls on the scores
        # dependency chain)
        xb_pool = ctx.enter_context(tc.tile_pool(name="xb", bufs=TPB // QUAD + 2))
        xt_pool = ctx.enter_context(tc.tile_pool(name="xt", bufs=6))
        ty_pool = ctx.enter_context(tc.tile_pool(name="ty", bufs=6))
        junk_pool = ctx.enter_context(tc.tile_pool(name="junk", bufs=4))
        fin_pool = ctx.enter_context(tc.tile_pool(name="fin", bufs=2))
        psy_pool = ctx.enter_context(tc.tile_pool(name="psy", bufs=3, space="PSUM"))
        pst_pool = ctx.enter_context(tc.tile_pool(name="pst", bufs=2, space="PSUM"))
        psn_pool = ctx.enter_context(tc.tile_pool(name="psn", bufs=2, space="PSUM"))
        psd_pool = ctx.enter_context(tc.tile_pool(name="psd", bufs=1, space="PSUM"))

        # ---- one-time setup ----
        # identity (cheap gpsimd ops, emitted before the x loads)
        ident = singles.tile([P, P], bf16)
        make_identity(nc, ident)

        # W in SBUF as [p, kt, e]: partition p of tile kt holds W[kt*128+p, :].
        # Loaded on the ACT HWDGE queue (parallel to the gpsimd x loads).
        w_f = singles.tile([P, KT, D], f32)
        nc.scalar.dma_start(out=w_f, in_=att_W.rearrange("(kt p) e -> p kt e", p=P))
        w_b = singles.tile([P, KT, D], bf16)
        nc.vector.tensor_copy(out=w_b, in_=w_f)

        # v broadcast across all 128 partitions: [128, 512]
        v_f = singles.tile([P, D], f32)
        v_ap = att_v[:]
        v_bcast = bass.AP(tensor=v_ap.tensor, offset=v_ap.offset,
                          ap=[[0, P]] + list(v_ap.ap))
        nc.scalar.dma_start(out=v_f, in_=v_bcast)
        v_b = singles.tile([P, D], bf16)
        nc.vector.tensor_copy(out=v_b, in_=v_f)

        ones_f = singles.tile([P, 1], f32)
        nc.vector.memset(ones_f, 1.0)

        scores = singles.tile([P, B_LOC, TPB], f32)
        e_b = singles.tile([P, B_LOC, TPB], bf16)

        for _rep in range(repeat or 1):
          for b in range(B_LOC):
            ps_num = psn_pool.tile([1, D], f32)
            # cast-DMA fp32 -> bf16, QUAD t-tiles per DMA (amortizes the ~1us
            # SWDGE descriptor-gen engine cost per dma_start)
            xquads = []
            for q in range(TPB // QUAD):
                xq = xb_pool.tile([P, QUAD, D], bf16)
                xquads.append(xq)
                if b == 0 and q == 0:
                    # split the first load so the pipeline ramps sooner
                    for qq in range(QUAD):
                        nc.gpsimd.dma_start(
                            out=xq[:, qq, :],
                            in_=x[b, qq * P:(qq + 1) * P, :])
                else:
                    nc.gpsimd.dma_start(
                        out=xq,
                        in_=x[b, q * QUAD * P:(q + 1) * QUAD * P, :].rearrange(
                            "(qq p) d -> p qq d", p=P))
            xbs = [xquads[i // QUAD][:, i % QUAD, :] for i in range(TPB)]
            for i in range(TPB):
                xb = xbs[i]
                # PE transpose: pst[p, j*128+t] = x[t0+t, j*128+p]  (bf16)
                pst = pst_pool.tile([P, KT * P], bf16)
                for j in range(KT):
                    nc.tensor.matmul(pst[:, j * P:(j + 1) * P],
                                     lhsT=xb[:, j * P:(j + 1) * P],
                                     rhs=ident, is_transpose=True)
                # evacuate PSUM -> SBUF (alternate engines to balance load)
                xt = xt_pool.tile([P, KT, P], bf16)
                xt_flat = xt.rearrange("p a c -> p (a c)")
                if i % 2 == 0:
                    nc.scalar.copy(out=xt_flat, in_=pst)
                else:
                    nc.vector.tensor_copy(out=xt_flat, in_=pst)
                # y tile = x_tile @ W : accumulate over 4 d-tiles
                psy = psy_pool.tile([P, D], f32)
                for j in range(KT):
                    nc.tensor.matmul(psy, lhsT=xt[:, j, :], rhs=w_b[:, j, :],
                                     start=(j == 0), stop=(j == KT - 1))
                # tanh -> SBUF bf16
                ty = ty_pool.tile([P, D], bf16)
                nc.scalar.activation(out=ty, in_=psy, func=AF.Tanh)
                # scores col = sum_e ty * v  (mult, then accumulate via a
                # tensor_scalar bypass op: 2x DVE mode vs 1x for TensorReduce)
                junk = junk_pool.tile([P, D], bf16)
                nc.vector.tensor_mul(junk, ty, v_b)
                junk2 = junk_pool.tile([P, D], bf16, tag="junk2")
                nc.vector.tensor_scalar(
                    out=junk2, in0=junk, scalar1=1.0, scalar2=0.0,
                    op0=ALU.mult, op1=ALU.add,
                    accum_out=scores[:, b, i:i + 1])
                # e col = exp(scores col), bf16 for the pooling matmul
                nc.scalar.activation(out=e_b[:, b, i:i + 1],
                                     in_=scores[:, b, i:i + 1], func=AF.Exp)

                # unnormalized pooling num += e_col^T @ x_tile, interleaved
                # with a lag so the PE never stalls on the scores chain
                if i >= POOL_LAG:
                    ip = i - POOL_LAG
                    nc.tensor.matmul(ps_num, lhsT=e_b[:, b, ip:ip + 1],
                                     rhs=xbs[ip], start=(ip == 0), stop=False,
                                     skip_group_check=True)

            for ip in range(TPB - POOL_LAG, TPB):
                nc.tensor.matmul(ps_num, lhsT=e_b[:, b, ip:ip + 1],
                                 rhs=xbs[ip], start=False,
                                 stop=(ip == TPB - 1), skip_group_check=True)

            # denominator: fresh fp32 exp of all scores of this batch
            e_f = fin_pool.tile([P, TPB], f32, tag="e_f")
            nc.scalar.activation(out=e_f, in_=scores[:, b, :], func=AF.Exp)
            part = fin_pool.tile([P, 1], f32, tag="part")
            nc.vector.tensor_reduce(out=part, in_=e_f,
                                    axis=mybir.AxisListType.X, op=ALU.add)
            ps_den = psd_pool.tile([1, 1], f32)
            nc.tensor.matmul(ps_den, lhsT=part, rhs=ones_f,
                             start=True, stop=True, skip_group_check=True)
            rec = fin_pool.tile([1, 1], f32, tag="rec")
            nc.vector.reciprocal(out=rec, in_=ps_den)
            # out row = num * (1/den)
            o_sb = fin_pool.tile([1, D], f32, tag="o_sb")
            nc.scalar.activation(out=o_sb, in_=ps_num, func=AF.Copy,
                                 scale=rec)
            nc.gpsimd.dma_start(out=out[b:b + 1, :], in_=o_sb)

    nc.compile()
    return nc


def _get_nc(repeat=None):
    key = ("nc", repeat)
    if key not in _cached:
        _cached[key] = _build(repeat)
    return _cached[key]


def kernel(x, att_W, att_v, trace=False):
    from concourse.bass_utils import run_bass_kernel_spmd

    x = np.ascontiguousarray(np.asarray(x, dtype=np.float32))
    att_W = np.ascontiguousarray(np.asarray(att_W, dtype=np.float32))
    att_v = np.ascontiguousarray(np.asarray(att_v, dtype=np.float32))

    nc = _get_nc()
    in_maps = [
        {"x": np.ascontiguousarray(x[c * B_LOC:(c + 1) * B_LOC]),
         "att_W": att_W, "att_v": att_v}
        for c in range(N_CORES)
    ]
    res = run_bass_kernel_spmd(nc, in_maps, core_ids=list(range(N_CORES)),
                               trace=trace)
    outs = [res.results[c]["out"] for c in range(N_CORES)]
    full = np.concatenate(outs, axis=0).astype(np.float32)
    if trace:
        return full, res
    return full


# revision 29
# speedup vs baseline: 1.1862x; 1.1862x over previous
"""Bass/Trainium2 kernel for additive-attention pooling.

    y = tanh(x @ W); s = y @ v; w = softmax(s, axis=T); out = w @ x

Shapes (full): x [16, 4096, 512] f32, att_W [512, 512] f32, att_v [512] f32
-> out [16, 512] f32.

Sharding: data-parallel over batch, 2 batches per core on 8 cores;
att_W / att_v replicated.

Per-core algorithm (streaming, single pass over x):
  for each 128-row tile of x:
    - GPSIMD cast-DMA: HBM fp32 -> SBUF bf16 (cast happens in the DMA)
    - PE transpose-mode matmuls put the D axis on partitions (PSUM bf16)
    - evacuate PSUM -> SBUF (alternating ScalarE/VectorE)
    - 4 accumulating bf16 matmuls vs resident W tiles -> y tile in PSUM
    - tanh on ScalarE (PSUM -> SBUF bf16)
    - scores col via fused multiply+reduce against broadcast v (DVE)
    - exp(scores col) on ScalarE (scores are ~N(0,0.05): no max needed)
    - accumulating M=1 matmul  e_col^T @ x_tile  -> unnormalized pooled sum
  per batch: den = sum(exp(scores)) via row-reduce + K=128 matmul with ones,
  out = num * (1/den).
"""

import numpy as np

B, T, D = 16, 4096, 512
N_CORES = 8
B_LOC = B // N_CORES          # batches per core
P = 128                       # partitions
TPB = T // P                  # 32 t-tiles per batch
KT = D // P                   # 4 contraction tiles
QUAD = 4                      # t-tiles per cast-DMA
POOL_LAG = 8                  # tiles of slack before a pooling matmul issues

_cached = {}


def _build(repeat=None):
    from contextlib import ExitStack

    import concourse.bass as bass
    import concourse.mybir as mybir
    from concourse import bacc
    from concourse.masks import make_identity
    from concourse.tile import TileContext

    f32 = mybir.dt.float32
    bf16 = mybir.dt.bfloat16
    AF = mybir.ActivationFunctionType
    ALU = mybir.AluOpType

    nc = bacc.Bacc("TRN2", target_bir_lowering=False, debug=False,
                   num_devices=N_CORES)
    x = nc.declare_dram_parameter("x", [B_LOC, T, D], f32, isOutput=False)
    att_W = nc.declare_dram_parameter("att_W", [D, D], f32, isOutput=False)
    att_v = nc.declare_dram_parameter("att_v", [D], f32, isOutput=False)
    out = nc.declare_dram_parameter("out", [B_LOC, D], f32, isOutput=True)

    with ExitStack() as ctx:
        tc = ctx.enter_context(TileContext(nc))
        singles = ctx.enter_context(tc.tile_pool(name="singles", bufs=1))
        # keep a whole batch of x tiles alive so the pooling matmuls can run
        # as one burst at batch end (no mid-stream PE stalls on the scores
        # dependency chain)
        xb_pool = ctx.enter_context(tc.tile_pool(name="xb", bufs=TPB // QUAD + 2))
        xt_pool = ctx.enter_context(tc.tile_pool(name="xt", bufs=6))
        ty_pool = ctx.enter_context(tc.tile_pool(name="ty", bufs=6))
        junk_pool = ctx.enter_context(tc.tile_pool(name="junk", bufs=4))
        fin_pool = ctx.enter_context(tc.tile_pool(name="fin", bufs=2))
        psy_pool = ctx.enter_context(tc.tile_pool(name="psy", bufs=3, space="PSUM"))
        pst_pool = ctx.enter_context(tc.tile_pool(name="pst", bufs=2, space="PSUM"))
        psn_pool = ctx.enter_context(tc.tile_pool(name="psn", bufs=2, space="PSUM"))
        psd_pool = ctx.enter_context(tc.tile_pool(name="psd", bufs=1, space="PSUM"))

        # ---- one-time setup ----
        # identity (cheap gpsimd ops, emitted before the x loads)
        ident = singles.tile([P, P], bf16)
        make_identity(nc, ident)

        # W in SBUF as [p, kt, e]: partition p of tile kt holds W[kt*128+p, :].
        # Loaded on the ACT HWDGE queue (parallel to the gpsimd x loads).
        w_f = singles.tile([P, KT, D], f32)
        nc.scalar.dma_start(out=w_f, in_=att_W.rearrange("(kt p) e -> p kt e", p=P))
        w_b = singles.tile([P, KT, D], bf16)
        nc.vector.tensor_copy(out=w_b, in_=w_f)

        # v broadcast across all 128 partitions: [128, 512]
        v_f = singles.tile([P, D], f32)
        v_ap = att_v[:]
        v_bcast = bass.AP(tensor=v_ap.tensor, offset=v_ap.offset,
                          ap=[[0, P]] + list(v_ap.ap))
        nc.scalar.dma_start(out=v_f, in_=v_bcast)
        v_b = singles.tile([P, D], bf16)
        nc.vector.tensor_copy(out=v_b, in_=v_f)

        ones_f = singles.tile([P, 1], f32)
        nc.vector.memset(ones_f, 1.0)

        scores = singles.tile([P, B_LOC, TPB], f32)
        e_b = singles.tile([P, B_LOC, TPB], bf16)

        for _rep in range(repeat or 1):
          for b in range(B_LOC):
            ps_num = psn_pool.tile([1, D], f32)
            # cast-DMA fp32 -> bf16, QUAD t-tiles per DMA (amortizes the ~1us
            # SWDGE descriptor-gen engine cost per dma_start)
            xquads = []
            for q in range(TPB // QUAD):
                xq = xb_pool.tile([P, QUAD, D], bf16)
                xquads.append(xq)
                if b == 0 and q == 0:
                    # split the first load so the pipeline ramps sooner
                    for qq in range(QUAD):
                        nc.gpsimd.dma_start(
                            out=xq[:, qq, :],
                            in_=x[b, qq * P:(qq + 1) * P, :])
                else:
                    nc.gpsimd.dma_start(
                        out=xq,
                        in_=x[b, q * QUAD * P:(q + 1) * QUAD * P, :].rearrange(
                            "(qq p) d -> p qq d", p=P))
            xbs = [xquads[i // QUAD][:, i % QUAD, :] for i in range(TPB)]
            for i in range(TPB):
                xb = xbs[i]
                # PE transpose: pst[p, j*128+t] = x[t0+t, j*128+p]  (bf16)
                pst = pst_pool.tile([P, KT * P], bf16)
                for j in range(KT):
                    nc.tensor.matmul(pst[:, j * P:(j + 1) * P],
                                     lhsT=xb[:, j * P:(j + 1) * P],
                                     rhs=ident, is_transpose=True)
                # evacuate PSUM -> SBUF (alternate engines to balance load)
                xt = xt_pool.tile([P, KT, P], bf16)
                xt_flat = xt.rearrange("p a c -> p (a c)")
                if i % 2 == 0:
                    nc.scalar.copy(out=xt_flat, in_=pst)
                else:
                    nc.vector.tensor_copy(out=xt_flat, in_=pst)
                # y tile = x_tile @ W : accumulate over 4 d-tiles
                psy = psy_pool.tile([P, D], f32)
                for j in range(KT):
                    nc.tensor.matmul(psy, lhsT=xt[:, j, :], rhs=w_b[:, j, :],
                                     start=(j == 0), stop=(j == KT - 1))
                # tanh -> SBUF bf16
                ty = ty_pool.tile([P, D], bf16)
                nc.scalar.activation(out=ty, in_=psy, func=AF.Tanh)
                # scores col = sum_e ty * v  (mult, then accumulate via a
                # tensor_scalar bypass op: 2x DVE mode vs 1x for TensorReduce)
                junk = junk_pool.tile([P, D], bf16)
                nc.vector.tensor_mul(junk, ty, v_b)
                junk2 = junk_pool.tile([P, D], bf16, tag="junk2")
                nc.vector.tensor_scalar(
                    out=junk2, in0=junk, scalar1=1.0, scalar2=0.0,
                    op0=ALU.mult, op1=ALU.add,
                    accum_out=scores[:, b, i:i + 1])
                # e col = exp(scores col), bf16 for the pooling matmul
                nc.scalar.activation(out=e_b[:, b, i:i + 1],
                                     in_=scores[:, b, i:i + 1], func=AF.Exp)

            # unnormalized pooling burst: num = sum_i e_col_i^T @ x_tile_i
            for ip in range(TPB):
                nc.tensor.matmul(ps_num, lhsT=e_b[:, b, ip:ip + 1],
                                 rhs=xbs[ip], start=(ip == 0),
                                 stop=(ip == TPB - 1), skip_group_check=True)

            # denominator: fresh fp32 exp of all scores of this batch
            e_f = fin_pool.tile([P, TPB], f32, tag="e_f")
            nc.scalar.activation(out=e_f, in_=scores[:, b, :], func=AF.Exp)
            part = fin_pool.tile([P, 1], f32, tag="part")
            nc.vector.tensor_reduce(out=part, in_=e_f,
                                    axis=mybir.AxisListType.X, op=ALU.add)
            ps_den = psd_pool.tile([1, 1], f32)
            nc.tensor.matmul(ps_den, lhsT=part, rhs=ones_f,
                             start=True, stop=True, skip_group_check=True)
            rec = fin_pool.tile([1, 1], f32, tag="rec")
            nc.vector.reciprocal(out=rec, in_=ps_den)
            # out row = num * (1/den)
            o_sb = fin_pool.tile([1, D], f32, tag="o_sb")
            nc.scalar.activation(out=o_sb, in_=ps_num, func=AF.Copy,
                                 scale=rec)
            nc.gpsimd.dma_start(out=out[b:b + 1, :], in_=o_sb)

    nc.compile()
    return nc


def _get_nc(repeat=None):
    key = ("nc", repeat)
    if key not in _cached:
        _cached[key] = _build(repeat)
    return _cached[key]


def kernel(x, att_W, att_v, trace=False):
    from concourse.bass_utils import run_bass_kernel_spmd

    x = np.ascontiguousarray(np.asarray(x, dtype=np.float32))
    att_W = np.ascontiguousarray(np.asarray(att_W, dtype=np.float32))
    att_v = np.ascontiguousarray(np.asarray(att_v, dtype=np.float32))

    nc = _get_nc()
    in_maps = [
        {"x": np.ascontiguousarray(x[c * B_LOC:(c + 1) * B_LOC]),
         "att_W": att_W, "att_v": att_v}
        for c in range(N_CORES)
    ]
    res = run_bass_kernel_spmd(nc, in_maps, core_ids=list(range(N_CORES)),
                               trace=trace)
    outs = [res.results[c]["out"] for c in range(N_CORES)]
    full = np.concatenate(outs, axis=0).astype(np.float32)
    if trace:
        return full, res
    return full
